# revision 21
# baseline (speedup 1.0000x reference)
"""AdaptGNN 3-layer message passing on 8 TRN2 NeuronCores.

Data-parallel over batch B=8: core c owns batch element c. Per core:
  h = x                                         [N=2048, D=128]
  for l in 0..2:
      hW   = h @ Wl + bl                        [N, 128]
      inv  = 1 / max(||hW||_row, eps)
      cos  = (inv inv^T) * (hW hW^T)            [N, N]
      h    = (ew * cos) @ hW                    [N, N] @ [N, 128]
      if l < 2: h = relu(h)

On-chip formulation (layouts chosen so no transposes of h are needed
beyond 16 PE tile-transposes per layer):
  - Loop state is hT_raw [128, N] (bf16, d on partitions) plus a deferred
    per-node scale s[n] (h_true[n,:] = s[n] * hT_raw[:,n]).
  - hW_nat tile (n-block t): matmul(lhsT=hT[:, t], rhs=W) -> psum, then
    scalar_tensor_tensor: (psum * s[t]) + b_bcast -> bf16. True hW values.
  - hWT via PE tile transpose of hW_nat.
  - Gram G[q,p] = hWT[:,q].T @ hWT[:,p] in psum; then
    MT[q,p] = ew[p,q] * inv[q] * G[q,p] via either
      (a) one DVE scalar_tensor_tensor (psum*inv)*ewT, or
      (b) ACT scaled-copy psum->sbuf bf16, then DVE/GPSIMD multiply,
    distributed to balance engine load.
  - aggT'[c,p] += hW_nat[q].T @ MT[q,p] accumulated over q in psum
    (= agg[p,c] / inv[p]; the missing inv[p] is the next layer's s).
  - next hT_raw = relu(aggT') (relu commutes with positive scale).
  - Final layer: transpose aggT' tiles to natural, multiply by inv[p]
    (per-partition), DMA out in f32.
  - ewT (transposed edge weights, bf16) built once: 4 SWDGE cast-DMAs
    f32->bf16 (512-column slabs) into DRAM scratch, then 16 HWDGE xbar
    transpose-DMAs into resident SBUF (64KB/partition).
"""

import functools

import numpy as np

N = 2048
D = 128
T = N // 128          # 16 row blocks
NCHUNK = N // 512     # 4 free-dim chunks for N=512 matmuls
N_CORES = 8
EPS = 1e-12


@functools.lru_cache(maxsize=1)
def build_nc():
    import concourse.bass as bass
    from concourse import bacc, masks, mybir, tile

    f32 = mybir.dt.float32
    bf16 = mybir.dt.bfloat16
    AF = mybir.ActivationFunctionType
    ALU = mybir.AluOpType

    nc = bacc.Bacc(None, target_bir_lowering=False)

    x_d = nc.declare_dram_parameter("x", [N, D], f32, isOutput=False)
    ew_d = nc.declare_dram_parameter("edge_weight", [N, N], f32, isOutput=False)
    w_d = []
    b_d = []
    for l in range(3):
        w_d.append(nc.declare_dram_parameter(f"W{l}", [D, D], f32, isOutput=False))
        b_d.append(nc.declare_dram_parameter(f"b{l}", [D], f32, isOutput=False))
    out_d = nc.declare_dram_parameter("out", [N, D], f32, isOutput=True)

    with tile.TileContext(nc) as tc:
        with (
            tc.tile_pool(name="persist", bufs=1) as persist,
            tc.tile_pool(name="consts", bufs=1) as consts,
            tc.tile_pool(name="hts", bufs=2) as hts,
            tc.tile_pool(name="hwn_p", bufs=2) as hwn_p,
            tc.tile_pool(name="hwt_p", bufs=2) as hwt_p,
            tc.tile_pool(name="inv_p", bufs=2) as inv_p,
            tc.tile_pool(name="scr_p", bufs=1) as scr_p,
            tc.tile_pool(name="gs_p", bufs=6) as gs_p,
            tc.tile_pool(name="mt_p", bufs=8) as mt_p,
            tc.tile_pool(name="lw_p", bufs=2) as lw_p,
            tc.tile_pool(name="psum", bufs=4, space="PSUM") as psum,
            tc.tile_pool(name="dram", bufs=4, space="DRAM") as dram,
        ):
            # ---- constants ----
            ident_f32 = consts.tile([128, 128], f32, tag="idf")
            ident_bf = consts.tile([128, 128], bf16, tag="idb")
            masks.make_identity(nc, ident_f32[:])
            masks.make_identity(nc, ident_bf[:])
            ones_row = consts.tile([1, 128], f32, tag="ones")
            nc.vector.memset(ones_row[:], 1.0)

            # ---- small loads: weights, biases, x ----
            W_bf = []
            b_bc = []
            for l in range(3):
                wb = consts.tile([128, 128], bf16, tag=f"wbf{l}", name=f"wb{l}")
                nc.gpsimd.dma_start(wb[:], w_d[l][:, :])  # cast f32->bf16
                W_bf.append(wb)
                brow = lw_p.tile([1, 128], f32, tag="brow", name=f"brow{l}")
                nc.sync.dma_start(brow[:], b_d[l].ap().rearrange("(o d) -> o d", o=1))
                bb = consts.tile([128, 128], f32, tag=f"bbc{l}", name=f"bb{l}")
                ps = psum.tile([128, 128], f32, tag="g", bufs=4)
                nc.tensor.matmul(ps[:], ones_row[:], brow[:])
                nc.scalar.activation(bb[:], ps[:], AF.Copy)
                b_bc.append(bb)

            # x natural (bf16 cast on load)
            xn = persist.tile([128, T, 128], bf16, tag="xn")
            nc.gpsimd.dma_start(
                xn[:], x_d.ap().rearrange("(t p) d -> p t d", p=128)
            )  # cast f32->bf16

            # ---- edge_weight: cast (col slabs) + 16 transposes ----
            ewT = persist.tile([128, T, N], bf16, tag="ewT")  # slab qb at [:, qb, :]
            slabs = [256, 256, 512, 512, 512]
            col0 = 0
            for s, w in enumerate(slabs):
                ewbf = dram.tile([N, w], bf16, tag=f"ewbf{s}", name=f"ewbf{s}")
                nc.gpsimd.dma_start(ewbf[:], ew_d[:, col0:col0 + w])
                for h in range(w // 128):
                    qb = (col0 // 128) + h
                    nc.sync.dma_start(
                        ewT[:, qb, :], ewbf[:, h * 128:(h + 1) * 128],
                        transpose=True,
                    )
                col0 += w

            # x transpose -> hT0
            hT = hts.tile([128, N], bf16, tag="hT")
            for t in range(T):
                ps = psum.tile([128, 128], bf16, tag="g", bufs=4)
                nc.tensor.transpose(ps[:], xn[:, t, :], ident_bf[:])
                nc.scalar.activation(hT[:, t * 128:(t + 1) * 128], ps[:], AF.Copy)

            # ---- layers ----
            for l in range(3):
                hwn = hwn_p.tile([128, T, 128], bf16, tag="hwn")
                nhT = hwt_p.tile([128, N], bf16, tag="nhT")
                n2 = inv_p.tile([128, T], f32, tag="n2")
                inv = inv_p.tile([128, T], f32, tag="inv")
                sq_scr = scr_p.tile([128, 128], f32, tag="sq")

                for t in range(T):
                    ps = psum.tile([128, 128], f32, tag="g", bufs=4)
                    nc.tensor.matmul(
                        ps[:], hT[:, t * 128:(t + 1) * 128], W_bf[l][:]
                    )
                    nc.vector.tensor_add(hwn[:, t, :], ps[:], b_bc[l][:])
                    nc.vector.tensor_mul(sq_scr[:], hwn[:, t, :], hwn[:, t, :])
                    nc.vector.reduce_sum(
                        n2[:, t:t + 1], sq_scr[:], axis=mybir.AxisListType.X
                    )

                # inv = 1 / max(sqrt(n2), eps)
                nrm = inv_p.tile([128, T], f32, tag="nrm")
                nc.scalar.activation(nrm[:], n2[:], AF.Sqrt)
                nc.vector.tensor_scalar_max(nrm[:], nrm[:], EPS)
                nc.vector.reciprocal(inv[:], nrm[:])

                # nh (normalized) tiles -> transpose -> nhT
                for t in range(T):
                    nh = scr_p.tile([128, 128], bf16, tag="nh", bufs=3)
                    nc.vector.tensor_scalar_mul(
                        nh[:], hwn[:, t, :], inv[:, t:t + 1]
                    )
                    ps2 = psum.tile([128, 128], bf16, tag="g", bufs=4)
                    nc.tensor.transpose(ps2[:], nh[:], ident_bf[:])
                    nc.scalar.activation(
                        nhT[:, t * 128:(t + 1) * 128], ps2[:], AF.Copy
                    )

                agg = [
                    psum.tile([128, 512], f32, tag="agg", name=f"agg{l}_{j}")
                    for j in range(NCHUNK)
                ]
                for qb in range(T):
                    for j in range(NCHUNK):
                        g_ps = psum.tile([128, 512], f32, tag="g", bufs=4)
                        nc.tensor.matmul(
                            g_ps[:],
                            nhT[:, qb * 128:(qb + 1) * 128],
                            nhT[:, j * 512:(j + 1) * 512],
                        )
                        ew_sl = ewT[:, qb, j * 512:(j + 1) * 512]
                        mt = mt_p.tile([128, 512], bf16, tag="mt")
                        if l > 0 and j == 0:
                            # fused: cos(psum) * ewT in one DVE op
                            nc.vector.tensor_tensor(
                                mt[:], g_ps[:], ew_sl, op=ALU.mult
                            )
                        else:
                            gs = gs_p.tile(
                                [128, 512], bf16, tag=f"gs{min(l, 1)}",
                                bufs=(36 if l == 0 else 8),
                            )
                            nc.scalar.activation(gs[:], g_ps[:], AF.Copy)
                            nc.vector.tensor_tensor(
                                mt[:], gs[:], ew_sl, op=ALU.mult
                            )
                        nc.tensor.matmul(
                            agg[j][:], hwn[:, qb, :], mt[:],
                            start=(qb == 0), stop=(qb == T - 1),
                        )

                if l < 2:
                    hT = hts.tile([128, N], bf16, tag="hT")
                    for j in range(NCHUNK):
                        nc.scalar.activation(
                            hT[:, j * 512:(j + 1) * 512], agg[j][:], AF.Relu
                        )
                else:
                    aggs = persist.tile([128, N], f32, tag="aggs")
                    for j in range(NCHUNK):
                        nc.scalar.activation(
                            aggs[:, j * 512:(j + 1) * 512], agg[j][:], AF.Copy
                        )
                    out_nat = persist.tile([128, T, 128], f32, tag="outn")
                    out_v = out_d.ap().rearrange("(t p) d -> p t d", p=128)
                    for t in range(T):
                        ps = psum.tile([128, 128], f32, tag="g", bufs=4)
                        nc.tensor.transpose(
                            ps[:], aggs[:, t * 128:(t + 1) * 128], ident_f32[:]
                        )
                        nc.scalar.activation(out_nat[:, t, :], ps[:], AF.Copy)
                        if t % 4 == 3:
                            nc.sync.dma_start(
                                out_v[:, t - 3:t + 1, :], out_nat[:, t - 3:t + 1, :]
                            )

    nc.compile()
    return nc


def kernel(**inputs):
    from concourse.bass_utils import run_bass_kernel_spmd

    x = np.asarray(inputs["x"], dtype=np.float32)
    ew = np.asarray(inputs["edge_weight"], dtype=np.float32)
    params = {}
    for l in range(3):
        params[f"W{l}"] = np.asarray(inputs[f"W{l}"], dtype=np.float32)
        params[f"b{l}"] = np.asarray(inputs[f"b{l}"], dtype=np.float32)

    nc = build_nc()
    in_maps = [
        {"x": x[c], "edge_weight": ew[c], **params} for c in range(N_CORES)
    ]
    res = run_bass_kernel_spmd(nc, in_maps, core_ids=list(range(N_CORES)))
    out = np.stack([res.results[c]["out"] for c in range(N_CORES)], axis=0)
    return out.astype(np.float32)


# revision 31
# speedup vs baseline: 1.1322x; 1.1322x over previous
"""AdaptGNN 3-layer message passing on 8 TRN2 NeuronCores.

Data-parallel over batch B=8: core c owns batch element c. Per core:
  h = x                                         [N=2048, D=128]
  for l in 0..2:
      hW   = h @ Wl + bl                        [N, 128]
      inv  = 1 / max(||hW||_row, eps)
      cos  = (inv inv^T) * (hW hW^T)            [N, N]
      h    = (ew * cos) @ hW                    [N, N] @ [N, 128]
      if l < 2: h = relu(h)

On-chip formulation (layouts chosen so no transposes of h are needed
beyond 16 PE tile-transposes per layer):
  - Loop state is hT_raw [128, N] (bf16, d on partitions) plus a deferred
    per-node scale s[n] (h_true[n,:] = s[n] * hT_raw[:,n]).
  - hW_nat tile (n-block t): matmul(lhsT=hT[:, t], rhs=W) -> psum, then
    scalar_tensor_tensor: (psum * s[t]) + b_bcast -> bf16. True hW values.
  - hWT via PE tile transpose of hW_nat.
  - Gram G[q,p] = hWT[:,q].T @ hWT[:,p] in psum; then
    MT[q,p] = ew[p,q] * inv[q] * G[q,p] via either
      (a) one DVE scalar_tensor_tensor (psum*inv)*ewT, or
      (b) ACT scaled-copy psum->sbuf bf16, then DVE/GPSIMD multiply,
    distributed to balance engine load.
  - aggT'[c,p] += hW_nat[q].T @ MT[q,p] accumulated over q in psum
    (= agg[p,c] / inv[p]; the missing inv[p] is the next layer's s).
  - next hT_raw = relu(aggT') (relu commutes with positive scale).
  - Final layer: transpose aggT' tiles to natural, multiply by inv[p]
    (per-partition), DMA out in f32.
  - ewT (transposed edge weights, bf16) built once: 4 SWDGE cast-DMAs
    f32->bf16 (512-column slabs) into DRAM scratch, then 16 HWDGE xbar
    transpose-DMAs into resident SBUF (64KB/partition).
"""

import functools

import numpy as np

N = 2048
D = 128
T = N // 128          # 16 row blocks
NCHUNK = N // 512     # 4 free-dim chunks for N=512 matmuls
N_CORES = 8
EPS = 1e-12


@functools.lru_cache(maxsize=1)
def build_nc():
    import concourse.bass as bass
    from concourse import bacc, masks, mybir, tile

    f32 = mybir.dt.float32
    bf16 = mybir.dt.bfloat16
    AF = mybir.ActivationFunctionType
    ALU = mybir.AluOpType

    nc = bacc.Bacc(None, target_bir_lowering=False)

    x_d = nc.declare_dram_parameter("x", [N, D], f32, isOutput=False)
    ew_d = nc.declare_dram_parameter("edge_weight", [N, N], f32, isOutput=False)
    w_d = []
    b_d = []
    for l in range(3):
        w_d.append(nc.declare_dram_parameter(f"W{l}", [D, D], f32, isOutput=False))
        b_d.append(nc.declare_dram_parameter(f"b{l}", [D], f32, isOutput=False))
    out_d = nc.declare_dram_parameter("out", [N, D], f32, isOutput=True)

    with tile.TileContext(nc) as tc:
        with (
            tc.tile_pool(name="persist", bufs=1) as persist,
            tc.tile_pool(name="consts", bufs=1) as consts,
            tc.tile_pool(name="hts", bufs=2) as hts,
            tc.tile_pool(name="hwn_p", bufs=2) as hwn_p,
            tc.tile_pool(name="hwt_p", bufs=2) as hwt_p,
            tc.tile_pool(name="inv_p", bufs=2) as inv_p,
            tc.tile_pool(name="scr_p", bufs=1) as scr_p,
            tc.tile_pool(name="gs_p", bufs=6) as gs_p,
            tc.tile_pool(name="mt_p", bufs=8) as mt_p,
            tc.tile_pool(name="lw_p", bufs=2) as lw_p,
            tc.tile_pool(name="psum", bufs=4, space="PSUM") as psum,
            tc.tile_pool(name="dram", bufs=4, space="DRAM") as dram,
        ):
            # ---- constants ----
            ident_f32 = consts.tile([128, 128], f32, tag="idf")
            ident_bf = consts.tile([128, 128], bf16, tag="idb")
            masks.make_identity(nc, ident_f32[:])
            masks.make_identity(nc, ident_bf[:])
            ones_row = consts.tile([1, 128], f32, tag="ones")
            nc.vector.memset(ones_row[:], 1.0)

            # ---- small loads: weights, biases, x ----
            W_bf = []
            b_bc = []
            for l in range(3):
                wb = consts.tile([128, 128], bf16, tag=f"wbf{l}", name=f"wb{l}")
                nc.gpsimd.dma_start(wb[:], w_d[l][:, :])  # cast f32->bf16
                W_bf.append(wb)
                brow = lw_p.tile([1, 128], f32, tag="brow", name=f"brow{l}")
                nc.sync.dma_start(brow[:], b_d[l].ap().rearrange("(o d) -> o d", o=1))
                bb = consts.tile([128, 128], f32, tag=f"bbc{l}", name=f"bb{l}")
                ps = psum.tile([128, 128], f32, tag="g", bufs=4)
                nc.tensor.matmul(ps[:], ones_row[:], brow[:])
                nc.scalar.activation(bb[:], ps[:], AF.Copy)
                b_bc.append(bb)

            # x natural (f32, via HWDGE so the SWDGE queue is free for ew)
            xn = persist.tile([128, T, 128], f32, tag="xn")
            nc.sync.dma_start(
                xn[:], x_d.ap().rearrange("(t p) d -> p t d", p=128)
            )

            # ---- edge_weight: cast (col slabs) + 16 transposes ----
            ewT = persist.tile([128, T, N], bf16, tag="ewT")  # slab qb at [:, qb, :]
            ewbf = dram.tile([N, N], bf16, tag="ewbf", name="ewbf")
            nc.gpsimd.dma_start(ewbf[:], ew_d[:, :])  # one contiguous 16MiB cast
            for qb in range(T):
                nc.sync.dma_start(
                    ewT[:, qb, :], ewbf[:, qb * 128:(qb + 1) * 128],
                    transpose=True,
                )

            # x transpose -> hT0 (f32 transpose, cast to bf16 on evac)
            hT = hts.tile([128, N], bf16, tag="hT")
            for t in range(T):
                ps = psum.tile([128, 128], f32, tag="g", bufs=4)
                nc.tensor.transpose(ps[:], xn[:, t, :], ident_f32[:])
                nc.scalar.activation(hT[:, t * 128:(t + 1) * 128], ps[:], AF.Copy)

            # ---- layers ----
            for l in range(3):
                hwn = hwn_p.tile([128, T, 128], bf16, tag="hwn")
                nhT = hwt_p.tile([128, N], bf16, tag="nhT")
                n2 = inv_p.tile([128, T], f32, tag="n2")
                inv = inv_p.tile([128, T], f32, tag="inv")
                sq_scr = scr_p.tile([128, 128], f32, tag="sq")

                for t in range(T):
                    ps = psum.tile([128, 128], f32, tag="g", bufs=4)
                    nc.tensor.matmul(
                        ps[:], hT[:, t * 128:(t + 1) * 128], W_bf[l][:]
                    )
                    nc.vector.tensor_add(hwn[:, t, :], ps[:], b_bc[l][:])
                    nc.vector.tensor_mul(sq_scr[:], hwn[:, t, :], hwn[:, t, :])
                    nc.vector.reduce_sum(
                        n2[:, t:t + 1], sq_scr[:], axis=mybir.AxisListType.X
                    )

                # inv = 1 / max(sqrt(n2), eps)
                nrm = inv_p.tile([128, T], f32, tag="nrm")
                nc.scalar.activation(nrm[:], n2[:], AF.Sqrt)
                nc.vector.tensor_scalar_max(nrm[:], nrm[:], EPS)
                nc.vector.reciprocal(inv[:], nrm[:])

                # nh (normalized) tiles -> transpose -> nhT
                for t in range(T):
                    nh = scr_p.tile([128, 128], bf16, tag="nh", bufs=3)
                    nc.vector.tensor_scalar_mul(
                        nh[:], hwn[:, t, :], inv[:, t:t + 1]
                    )
                    ps2 = psum.tile([128, 128], bf16, tag="g", bufs=4)
                    nc.tensor.transpose(ps2[:], nh[:], ident_bf[:])
                    nc.scalar.activation(
                        nhT[:, t * 128:(t + 1) * 128], ps2[:], AF.Copy
                    )

                agg = [
                    psum.tile([128, 512], f32, tag="agg", name=f"agg{l}_{j}")
                    for j in range(NCHUNK)
                ]
                for qb in range(T):
                    for j in range(NCHUNK):
                        g_ps = psum.tile([128, 512], f32, tag="g", bufs=4)
                        nc.tensor.matmul(
                            g_ps[:],
                            nhT[:, qb * 128:(qb + 1) * 128],
                            nhT[:, j * 512:(j + 1) * 512],
                        )
                        ew_sl = ewT[:, qb, j * 512:(j + 1) * 512]
                        mt = mt_p.tile([128, 512], bf16, tag="mt")
                        if l > 0 and j == 0:
                            # fused: cos(psum) * ewT in one DVE op
                            nc.vector.tensor_tensor(
                                mt[:], g_ps[:], ew_sl, op=ALU.mult
                            )
                        else:
                            gs = gs_p.tile(
                                [128, 512], bf16, tag=f"gs{min(l, 1)}",
                                bufs=(48 if l == 0 else 8),
                            )
                            nc.scalar.activation(gs[:], g_ps[:], AF.Copy)
                            nc.vector.tensor_tensor(
                                mt[:], gs[:], ew_sl, op=ALU.mult
                            )
                        nc.tensor.matmul(
                            agg[j][:], hwn[:, qb, :], mt[:],
                            start=(qb == 0), stop=(qb == T - 1),
                        )

                if l < 2:
                    hT = hts.tile([128, N], bf16, tag="hT")
                    for j in range(NCHUNK):
                        nc.scalar.activation(
                            hT[:, j * 512:(j + 1) * 512], agg[j][:], AF.Relu
                        )
                else:
                    aggs = persist.tile([128, N], f32, tag="aggs")
                    for j in range(NCHUNK):
                        nc.scalar.activation(
                            aggs[:, j * 512:(j + 1) * 512], agg[j][:], AF.Copy
                        )
                    out_nat = persist.tile([128, T, 128], f32, tag="outn")
                    for t in range(T):
                        ps = psum.tile([128, 128], f32, tag="g", bufs=4)
                        nc.tensor.transpose(
                            ps[:], aggs[:, t * 128:(t + 1) * 128], ident_f32[:]
                        )
                        nc.scalar.activation(out_nat[:, t, :], ps[:], AF.Copy)
                    nc.sync.dma_start(
                        out_d.ap().rearrange("(t p) d -> p t d", p=128), out_nat[:]
                    )

    nc.compile()
    return nc


def kernel(**inputs):
    from concourse.bass_utils import run_bass_kernel_spmd

    x = np.asarray(inputs["x"], dtype=np.float32)
    ew = np.asarray(inputs["edge_weight"], dtype=np.float32)
    params = {}
    for l in range(3):
        params[f"W{l}"] = np.asarray(inputs[f"W{l}"], dtype=np.float32)
        params[f"b{l}"] = np.asarray(inputs[f"b{l}"], dtype=np.float32)

    nc = build_nc()
    in_maps = [
        {"x": x[c], "edge_weight": ew[c], **params} for c in range(N_CORES)
    ]
    res = run_bass_kernel_spmd(nc, in_maps, core_ids=list(range(N_CORES)))
    out = np.stack([res.results[c]["out"] for c in range(N_CORES)], axis=0)
    return out.astype(np.float32)
